# revision 23
# baseline (speedup 1.0000x reference)
"""PerceiverAR attention kernel for 8 Trainium2 NeuronCores.

Sharding: (batch=2) x (head-groups=4) -> 8 cores. Each core computes, for its
batch b and its 4 heads, the full attention output projected through its slice
of Wo (row-parallel partial sum). Host sums the 4 partials per batch and adds bo.

Per-core pipeline (all on one NeuronCore, Tile framework):
  - kv proj (x^T stationary, f32r matmuls), RoPE, per-head LayerNorm
  - q proj (latent rows only), RoPE (1/sqrt(dh) folded into cos/sin), gamma fold
  - scores computed transposed (keys on partitions) so exp output feeds the
    AV matmul directly as the moving operand; softmax denominator comes from an
    appended ones-column in kv (prob-sum trick), applied after AV.
  - causal masking of diagonal 128-blocks via affine_select after exp (gpsimd)
  - rstd computed as exp(-0.5*ln(var+eps)) so ACT only ever uses one table set
  - emission is software-pipelined: projection chunk-groups interleave with
    attention blocks (engines run their streams in order, so emission order
    controls overlap); attention runs in 4 waves of 2 (b,h)-streams to fit
    the 8 PSUM banks (2x2-bank score blocks, 2x1-bank AV accumulators,
    2x1-bank misc for proj/transpose/wout).
"""

import os
import sys

if os.environ.get("JAX_PLATFORMS", "").strip().lower() in ("cpu",):
    # kernel needs the axon PJRT backend; a cpu pin would hide the neuron cores
    os.environ.pop("JAX_PLATFORMS")

sys.path.insert(0, "/opt/trn_rl_repo")

import numpy as np
from contextlib import ExitStack

import concourse.bass as bass
import concourse.tile as tile
from concourse import bacc, mybir
from concourse.masks import make_identity

F32 = mybir.dt.float32
F32R = mybir.dt.float32r

DIM = 1024
HEADS = 16
DH = 64
SEQ = 4096
LAT = 1024
HIST = SEQ - LAT  # 3072
LN_EPS = 1e-5

NCORES = 8
HL = 4              # heads per core
E = HL * DH         # 256 local projection cols
NCH = SEQ // 128    # 32 key chunks
DC = DIM // 128     # 8 contraction chunks
QCH0 = HIST // 128  # 24: first latent n-chunk
NG = 2              # 512-wide latent column groups per head
JLIM = (28, 32)     # key chunks per group (3584 and 4096 keys)

LAST_RESULTS = None
LAST_EXEC_NS = None


def _rr(ap, f32r=True):
    return ap.bitcast(F32R) if f32r else ap


def _mm(nc, out, lhsT, rhs, start, stop, f32r=True):
    if f32r:
        lhsT = lhsT.bitcast(F32R)
        rhs = rhs.bitcast(F32R)
    nc.tensor.matmul(out, lhsT, rhs, start=start, stop=stop)


def _bcast4(ap):
    """[128, 64] sbuf AP -> [128, 4, 64] stride-0 broadcast over head dim."""
    return bass.AP(
        tensor=ap.tensor,
        offset=ap.offset,
        ap=[list(ap.ap[0]), [0, HL], list(ap.ap[-1])],
    )


def build_nc(fold_gb=True, f32r=True):
    nc = bacc.Bacc("TRN2", target_bir_lowering=False, debug=False)

    xt = nc.declare_dram_parameter("xt", [NCH, 128, DC, 128], F32, isOutput=False)
    wq_t = nc.declare_dram_parameter("wq_t", [DIM, E], F32, isOutput=False)
    wkv_t = nc.declare_dram_parameter("wkv_t", [DIM, E], F32, isOutput=False)
    wo_ct = nc.declare_dram_parameter("wo_ct", [E, DIM], F32, isOutput=False)
    cos_k = nc.declare_dram_parameter("cos_k", [SEQ, DH], F32, isOutput=False)
    sin_k = nc.declare_dram_parameter("sin_k", [SEQ, DH], F32, isOutput=False)
    cos_q = nc.declare_dram_parameter("cos_q", [LAT, DH], F32, isOutput=False)
    sin_q = nc.declare_dram_parameter("sin_q", [LAT, DH], F32, isOutput=False)
    gamma = nc.declare_dram_parameter("gamma", [DH], F32, isOutput=False)
    beta = nc.declare_dram_parameter("beta", [DH], F32, isOutput=False)
    y = nc.declare_dram_parameter("y", [LAT, DIM], F32, isOutput=True)


    with tile.TileContext(nc) as tc:
        with ExitStack() as ctx:
            singles = ctx.enter_context(tc.tile_pool(name="singles", bufs=1))
            kvnp = ctx.enter_context(tc.tile_pool(name="kvn", bufs=NCH))
            kvtp = ctx.enter_context(tc.tile_pool(name="kvt", bufs=2))
            qtp = ctx.enter_context(tc.tile_pool(name="qt", bufs=2))
            otp = ctx.enter_context(tc.tile_pool(name="ot", bufs=2))
            xsp = ctx.enter_context(tc.tile_pool(name="xs", bufs=3))
            csp = ctx.enter_context(tc.tile_pool(name="cs", bufs=6))
            tmpp = ctx.enter_context(tc.tile_pool(name="tmp", bufs=2))
            statp = ctx.enter_context(tc.tile_pool(name="stat", bufs=8))
            mvp = ctx.enter_context(tc.tile_pool(name="mvp", bufs=3))
            kvrp = ctx.enter_context(tc.tile_pool(name="kvr", bufs=3))
            kvcp = ctx.enter_context(tc.tile_pool(name="kvc", bufs=3))
            expp = ctx.enter_context(tc.tile_pool(name="expp", bufs=3))
            rsp = ctx.enter_context(tc.tile_pool(name="rs", bufs=1))
            rbp = ctx.enter_context(tc.tile_pool(name="rb", bufs=1))
            ysp = ctx.enter_context(tc.tile_pool(name="ys", bufs=2))
            # PSUM: 2x2 (scores) + 2x1 (AV accumulators) + 2x1 (misc) = 8 banks
            psc = ctx.enter_context(tc.tile_pool(name="pp_sc", bufs=2, space="PSUM"))
            pav = ctx.enter_context(tc.tile_pool(name="pp_av", bufs=2, space="PSUM"))
            pmisc = ctx.enter_context(tc.tile_pool(name="pp_misc", bufs=2, space="PSUM"))

            # ---- constants ----
            ident = singles.tile([128, 128], F32)
            make_identity(nc, ident[:, :])
            wkv_sb = singles.tile([128, DC, E], F32)
            nc.sync.dma_start(out=_rr(wkv_sb[:, :, :], f32r),
                              in_=_rr(wkv_t.rearrange("(dc p) e -> p dc e", p=128), f32r))
            wq_sb = singles.tile([128, DC, E], F32)
            nc.sync.dma_start(out=_rr(wq_sb[:, :, :], f32r),
                              in_=_rr(wq_t.rearrange("(dc p) e -> p dc e", p=128), f32r))
            wo_sb = singles.tile([128, 2, DIM], F32)
            nc.sync.dma_start(out=_rr(wo_sb[:, :, :], f32r),
                              in_=_rr(wo_ct.rearrange("(pc p) f -> p pc f", p=128), f32r))
            eps_t = singles.tile([128, 1], F32)
            nc.vector.memset(eps_t[:, :], LN_EPS)
            ones64 = singles.tile([1, 64], F32)
            nc.vector.memset(ones64[:, :], 1.0)
            gam_b = singles.tile([128, E], F32)
            g_ap = gamma[:]
            nc.sync.dma_start(
                out=gam_b[:, :].rearrange("p (h d) -> p h d", h=HL),
                in_=bass.AP(tensor=g_ap.tensor, offset=g_ap.offset, ap=[[0, 128], [0, HL], [1, DH]]),
            )
            bet_b = None
            if not fold_gb:
                bet_b = singles.tile([128, E], F32)
                b_ap = beta[:]
                nc.sync.dma_start(
                    out=bet_b[:, :].rearrange("p (h d) -> p h d", h=HL),
                    in_=bass.AP(tensor=b_ap.tensor, offset=b_ap.offset, ap=[[0, 128], [0, HL], [1, DH]]),
                )
            gcol = singles.tile([128, 1], F32)
            bcol = singles.tile([128, 1], F32)
            b_ap = beta[:]
            for half in range(2):
                nc.sync.dma_start(
                    out=gcol[half * DH:(half + 1) * DH, :],
                    in_=bass.AP(tensor=g_ap.tensor, offset=g_ap.offset, ap=[[1, DH], [0, 1]]),
                )
                nc.sync.dma_start(
                    out=bcol[half * DH:(half + 1) * DH, :],
                    in_=bass.AP(tensor=b_ap.tensor, offset=b_ap.offset, ap=[[1, DH], [0, 1]]),
                )

            # persistent per-head-pair tensors
            kvt_sb = [kvtp.tile([128, SEQ], F32, name=f"kvt{_p}", tag=f"kvt{_p}") for _p in range(2)]
            qt_sb = [qtp.tile([128, LAT], F32, name=f"qt{_p}", tag=f"qt{_p}") for _p in range(2)]
            ot_sb = [otp.tile([128, LAT], F32, name=f"ot{_p}", tag=f"ot{_p}") for _p in range(2)]

            kvn_by_chunk = [None] * NCH
            kvc_by_chunk = [None] * NCH
            p_av_by_stream = {}

            def rope(dst3, src_ap, cos_sl, sin_sl, tmp):
                """dst3: [128, 4, 64] view; src: [128, 256] SBUF AP; cos/sin: [128, 64]."""
                tmp3 = tmp[:, :].rearrange("p (h s w) -> p h s w", h=HL, s=2)
                swap = bass.AP(
                    tensor=src_ap.tensor,
                    offset=src_ap.offset + 32,
                    ap=[list(src_ap.ap[0]), [DH, HL], [-32, 2], [1, 32]],
                )
                sin_b = _bcast4(sin_sl)
                sin_b4 = bass.AP(tensor=sin_b.tensor, offset=sin_b.offset,
                                 ap=[list(sin_b.ap[0]), [0, HL], [32, 2], [1, 32]])
                nc.vector.tensor_mul(tmp3, swap, sin_b4)
                nc.vector.tensor_mul(dst3, src_ap.rearrange("p (h d) -> p h d", h=HL), _bcast4(cos_sl))
                nc.gpsimd.tensor_add(dst3, dst3, tmp[:, :].rearrange("p (h d) -> p h d", h=HL))

            def proj_group(chunks, copies_on_act):
                """Project 4 n-chunks: kv (+q for latent chunks), RoPE, LN."""
                c0 = chunks[0]
                cosk_g = csp.tile([128, 4, DH], F32, name="cosk_g", tag="cs")
                nc.sync.dma_start(out=cosk_g[:, :, :],
                                  in_=cos_k.rearrange("(c p) d -> p c d", p=128)[:, c0:c0 + 4, :])
                sink_g = csp.tile([128, 4, DH], F32, name="sink_g", tag="cs")
                nc.sync.dma_start(out=sink_g[:, :, :],
                                  in_=sin_k.rearrange("(c p) d -> p c d", p=128)[:, c0:c0 + 4, :])
                if c0 >= QCH0:
                    cosq_g = csp.tile([128, 4, DH], F32, name="cosq_g", tag="cs")
                    nc.sync.dma_start(out=cosq_g[:, :, :],
                                      in_=cos_q.rearrange("(c p) d -> p c d", p=128)[:, c0 - QCH0:c0 - QCH0 + 4, :])
                    sinq_g = csp.tile([128, 4, DH], F32, name="sinq_g", tag="cs")
                    nc.sync.dma_start(out=sinq_g[:, :, :],
                                      in_=sin_q.rearrange("(c p) d -> p c d", p=128)[:, c0 - QCH0:c0 - QCH0 + 4, :])
                for sub0 in range(0, len(chunks), 2):
                  sub = chunks[sub0:sub0 + 2]
                  mv4 = mvp.tile([128, 2, HL, 2], F32, name="mv4", tag="mv4")
                  rstd4 = mvp.tile([128, 2, HL], F32, name="rstd4", tag="rstd4")
                  for ci, nch in enumerate(sub):
                    ci_g = sub0 + ci
                    xt_t = xsp.tile([128, DC, 128], F32, name="xt_t", tag="xt")
                    nc.sync.dma_start(out=_rr(xt_t[:, :, :], f32r), in_=_rr(xt[nch, :, :, :], f32r))
                    p_kv = pmisc.tile([128, E], F32, name="p_kv", tag="misc")
                    for dc in range(DC):
                        _mm(nc, p_kv[:, :], xt_t[:, dc, :], wkv_sb[:, dc, :],
                            start=(dc == 0), stop=(dc == DC - 1), f32r=f32r)
                    # RoPE into a contiguous working tile; kvn (65-stride + ones)
                    # is filled later by one strided copy.
                    kvn_t = kvnp.tile([128, HL * 65], F32, name="kvn_t", tag="kvn")
                    kvn_by_chunk[nch] = kvn_t
                    kvn3 = kvn_t[:, :].rearrange("p (h e) -> p h e", e=65)
                    kvraw = kvrp.tile([128, E], F32, name="kvraw", tag="kvr")
                    nc.vector.tensor_copy(kvraw[:, :], p_kv[:, :])
                    kvc = kvcp.tile([128, E], F32, name="kvc", tag="kvc")
                    kvc_by_chunk[nch] = kvc
                    tmp = tmpp.tile([128, E], F32, name="tmp", tag="tmp")
                    rope(kvc[:, :].rearrange("p (h d) -> p h d", h=HL), kvraw[:, :],
                         cosk_g[:, ci_g, :], sink_g[:, ci_g, :], tmp)
                    eap = eps_t[:, :]
                    nc.vector.tensor_scalar(
                        out=_rr(kvn3[:, :, DH:65], f32r),
                        in0=bass.AP(tensor=eap.tensor, offset=eap.offset,
                                    ap=[list(eap.ap[0]), [0, HL], [0, 1]]),
                        scalar1=0.0, scalar2=1.0,
                        op0=mybir.AluOpType.mult, op1=mybir.AluOpType.add,
                    )
                    for hh in range(HL):
                        st = statp.tile([128, 6], F32, name="st", tag="st")
                        nc.vector.bn_stats(out=st[:, :], in_=kvc[:, hh * DH:(hh + 1) * DH])
                        nc.vector.bn_aggr(out=mv4[:, ci, hh, :], in_=st[:, :])
                    if nch >= QCH0:
                        nq = nch - QCH0
                        p_q = pmisc.tile([128, E], F32, name="p_q", tag="misc")
                        for dc in range(DC):
                            _mm(nc, p_q[:, :], xt_t[:, dc, :], wq_sb[:, dc, :],
                                start=(dc == 0), stop=(dc == DC - 1), f32r=f32r)
                        q_ro = kvrp.tile([128, E], F32, name="q_ro", tag="kvr")
                        nc.vector.tensor_copy(q_ro[:, :], p_q[:, :])
                        tmpq = tmpp.tile([128, E], F32, name="tmpq", tag="tmp")
                        rope(q_ro[:, :].rearrange("p (h d) -> p h d", h=HL), q_ro[:, :],
                             cosq_g[:, ci_g, :], sinq_g[:, ci_g, :], tmpq)
                        if fold_gb:
                            nc.vector.tensor_mul(q_ro[:, :], q_ro[:, :], gam_b[:, :])
                        p_tr = pmisc.tile([128, 256], F32, name="p_tr", tag="misc")
                        for p in range(2):
                            nc.tensor.transpose(p_tr[:, 128 * p:128 * p + 128],
                                                q_ro[:, 128 * p:128 * p + 128], ident[:, :])
                        for p in range(2):
                            nc.scalar.copy(_rr(qt_sb[p][:, nq * 128:(nq + 1) * 128], f32r),
                                           p_tr[:, 128 * p:128 * p + 128])
                  # rstd = exp(-0.5*ln(var+eps)) for the subgroup in 2 ACT ops
                  nc.scalar.activation(out=rstd4[:, :, :], in_=mv4[:, :, :, 1],
                                       func=mybir.ActivationFunctionType.Ln,
                                       bias=eps_t[:, :], scale=1.0)
                  nc.scalar.activation(out=rstd4[:, :, :], in_=rstd4[:, :, :],
                                       func=mybir.ActivationFunctionType.Exp, scale=-0.5)
                  for ci, nch in enumerate(sub):
                    kvn_t = kvn_by_chunk[nch]
                    kvc = kvc_by_chunk[nch]
                    kvn3 = kvn_t[:, :].rearrange("p (h e) -> p h e", e=65)
                    for hh in range(HL):
                        nc.gpsimd.tensor_scalar(
                            out=kvc[:, hh * DH:(hh + 1) * DH], in0=kvc[:, hh * DH:(hh + 1) * DH],
                            scalar1=mv4[:, ci, hh, 0:1], scalar2=rstd4[:, ci, hh:hh + 1],
                            op0=mybir.AluOpType.subtract, op1=mybir.AluOpType.mult,
                        )
                        if not fold_gb:
                            nc.vector.tensor_mul(kvc[:, hh * DH:(hh + 1) * DH], kvc[:, hh * DH:(hh + 1) * DH],
                                                 gam_b[:, hh * DH:(hh + 1) * DH])
                            nc.vector.tensor_add(kvc[:, hh * DH:(hh + 1) * DH], kvc[:, hh * DH:(hh + 1) * DH],
                                                 bet_b[:, hh * DH:(hh + 1) * DH])
                    nc.vector.tensor_copy(_rr(kvn3[:, :, 0:DH], f32r),
                                          kvc[:, :].rearrange("p (h d) -> p h d", h=HL))
                    p_tr = pmisc.tile([128, 256], F32, name="p_tr2", tag="misc")
                    for p in range(2):
                        nc.tensor.transpose(p_tr[:, 128 * p:128 * p + 128],
                                            kvc[:, 128 * p:128 * p + 128], ident[:, :])
                    for p in range(2):
                        dst = _rr(kvt_sb[p][:, nch * 128:(nch + 1) * 128], f32r)
                        if p == 1 and copies_on_act:
                            nc.scalar.copy(dst, p_tr[:, 128 * p:128 * p + 128])
                        else:
                            nc.vector.tensor_copy(dst, p_tr[:, 128 * p:128 * p + 128])

            def attn_scores_exp(hh, g, k):
                p = hh // 2
                r0 = 64 * (hh % 2)
                jlim = JLIM[g]
                p_sc = psc.tile([128, 1024], F32, name="p_sc", tag="sc")
                for u, j in enumerate((2 * k, 2 * k + 1)):
                    _mm(nc, p_sc[:, u * 512:(u + 1) * 512],
                        kvt_sb[p][r0:r0 + DH, j * 128:(j + 1) * 128],
                        qt_sb[p][r0:r0 + DH, g * 512:(g + 1) * 512],
                        start=True, stop=True, f32r=f32r)
                exp_t = expp.tile([128, 1024], F32, name="exp_t", tag="exp")
                nc.scalar.activation(out=_rr(exp_t[:, :], f32r), in_=p_sc[:, :],
                                     func=mybir.ActivationFunctionType.Exp)
                for u, j in enumerate((2 * k, 2 * k + 1)):
                    t = j - (jlim - 4)
                    if t >= 0:
                        lo = u * 512
                        nc.gpsimd.affine_select(
                            out=_rr(exp_t[:, lo:lo + 512], f32r), in_=exp_t[:, lo:lo + 512],
                            compare_op=mybir.AluOpType.is_ge,
                            fill=0.0, base=-128 * t, channel_multiplier=-1,
                            pattern=[[1, 512]],
                        )
                return exp_t

            def attn_av(hh, g, k, exp_t):
                jlim = JLIM[g]
                key = (hh, g)
                if key not in p_av_by_stream:
                    p_av_by_stream[key] = pav.tile([65, 512], F32, name=f"p_av{hh}{g}", tag="av")
                p_av = p_av_by_stream[key]
                for u, j in enumerate((2 * k, 2 * k + 1)):
                    kvn_t = kvn_by_chunk[j]
                    _mm(nc, p_av[:, :],
                        kvn_t[:, :].rearrange("p (h e) -> p h e", e=65)[:, hh, :],
                        exp_t[:, u * 512:(u + 1) * 512],
                        start=(j == 24), stop=(j == 23), f32r=f32r)

            def attn_norm(hh, g):
                p = hh // 2
                r0 = 64 * (hh % 2)
                p_av = p_av_by_stream.pop((hh, g))
                rs = rsp.tile([1, 512], F32, name="rs", tag="rs")
                nc.vector.reciprocal(rs[:, :], p_av[64:65, :])
                p_rb = pmisc.tile([64, 512], F32, name="p_rb", tag="misc")
                nc.tensor.matmul(p_rb[:, :], ones64[:, :], rs[:, :], start=True, stop=True)
                rb = rbp.tile([64, 512], F32, name="rb", tag="rb")
                nc.vector.tensor_copy(rb[:, :], p_rb[:, :])
                oslice = ot_sb[p][r0:r0 + DH, g * 512:(g + 1) * 512]
                nc.vector.tensor_mul(_rr(oslice, f32r), p_av[0:DH, :], rb[:, :])
                if fold_gb:
                    nc.vector.tensor_scalar(
                        out=_rr(oslice, f32r), in0=oslice,
                        scalar1=gcol[r0:r0 + DH, :], scalar2=bcol[r0:r0 + DH, :],
                        op0=mybir.AluOpType.mult, op1=mybir.AluOpType.add,
                    )

            # ---------------- emission (software-pipelined) ----------------
            groups = [
                list(range(24, 28)), list(range(28, 32)),
                [0, 1, 2, 3], [4, 5, 6, 7], [8, 9, 10, 11],
                [12, 13, 14, 15], [16, 17, 18, 19], [20, 21, 22, 23],
            ]
            proj_group(groups[0], copies_on_act=True)
            proj_group(groups[1], copies_on_act=True)
            pending = groups[2:]

            ks = [12, 13, 14, 15] + list(range(12))
            waves = [[(0, 0), (0, 1)], [(1, 0), (1, 1)], [(2, 0), (2, 1)], [(3, 0), (3, 1)]]
            for w, wave in enumerate(waves):
                for pos, k in enumerate(ks):
                    if w == 0 and pos % 2 == 0 and pending:
                        proj_group(pending.pop(0), copies_on_act=False)
                    exps = []
                    for (hh, g) in wave:
                        if k < JLIM[g] // 2:
                            exps.append((hh, g, attn_scores_exp(hh, g, k)))
                    for (hh, g, exp_t) in exps:
                        attn_av(hh, g, k, exp_t)
                for (hh, g) in wave:
                    attn_norm(hh, g)

            # ---------------- output projection ----------------
            for ic in range(LAT // 128):
                y_sb = ysp.tile([128, DIM], F32, name="y_sb", tag="ys")
                for fh in range(2):
                    if fh == 0:
                        p_y = pmisc.tile([128, 512], F32, name="p_y", tag="misc")
                    else:
                        p_y = psc.tile([128, 512], F32, name="p_y2", tag="sc")
                    for p in range(2):
                        _mm(nc, p_y[:, :],
                            ot_sb[p][:, ic * 128:(ic + 1) * 128],
                            wo_sb[:, p, fh * 512:(fh + 1) * 512],
                            start=(p == 0), stop=(p == 1), f32r=f32r)
                    if fh == 0:
                        nc.vector.tensor_copy(y_sb[:, 0:512], p_y[:, :])
                    else:
                        nc.scalar.copy(y_sb[:, 512:1024], p_y[:, :])
                nc.sync.dma_start(out=y[ic * 128:(ic + 1) * 128, :], in_=y_sb[:, :])
    nc.finalize()
    return nc


def make_core_inputs(x, Wq, Wkv, Wo, gamma, beta):
    """Build the 8 per-core input maps (host-side shard prep)."""
    inv_freq = 1.0 / (10000.0 ** (np.arange(0, DH, 2, dtype=np.float32) / DH))
    freqs = np.arange(SEQ, dtype=np.float32)[:, None] * inv_freq[None, :]
    emb = np.concatenate([freqs, freqs], axis=-1)
    cos_k = np.cos(emb).astype(np.float32)
    # sign-baked sin for the "swapped-halves" rope formulation
    sin_k = np.concatenate([-np.sin(freqs), np.sin(freqs)], axis=-1).astype(np.float32)
    s = np.float32(DH ** -0.5)
    cos_q = (cos_k[HIST:] * s).astype(np.float32)
    sin_q = (sin_k[HIST:] * s).astype(np.float32)

    maps = []
    for c in range(NCORES):
        b = c // 4
        hg = c % 4
        rows = slice(hg * E, (hg + 1) * E)
        maps.append({
            "xt": np.ascontiguousarray(
                x[b].T.reshape(DC, 128, NCH, 128).transpose(2, 1, 0, 3), dtype=np.float32),
            "wq_t": np.ascontiguousarray(Wq[rows].T, dtype=np.float32),
            "wkv_t": np.ascontiguousarray(Wkv[rows].T, dtype=np.float32),
            "wo_ct": np.ascontiguousarray(Wo[:, rows].T, dtype=np.float32),
            "cos_k": cos_k, "sin_k": sin_k, "cos_q": cos_q, "sin_q": sin_q,
            "gamma": np.asarray(gamma, dtype=np.float32),
            "beta": np.asarray(beta, dtype=np.float32),
        })
    return maps


_NC_CACHE = {}


def kernel(x, Wq, Wkv, Wo, bo, gamma, beta):
    global LAST_RESULTS, LAST_EXEC_NS
    x = np.asarray(x, dtype=np.float32)
    Wq = np.asarray(Wq, dtype=np.float32)
    Wkv = np.asarray(Wkv, dtype=np.float32)
    Wo = np.asarray(Wo, dtype=np.float32)
    bo = np.asarray(bo, dtype=np.float32)
    gamma = np.asarray(gamma, dtype=np.float32)
    beta = np.asarray(beta, dtype=np.float32)

    fold_gb = bool(np.all(beta == 0.0))
    f32r = os.environ.get("KERNEL_MM_F32R", "1") == "1"
    key = (fold_gb, f32r)
    if key not in _NC_CACHE:
        _NC_CACHE[key] = build_nc(fold_gb=fold_gb, f32r=f32r)
    nc = _NC_CACHE[key]

    from concourse.bass_utils import run_bass_kernel_spmd

    in_maps = make_core_inputs(x, Wq, Wkv, Wo, gamma, beta)
    trace = os.environ.get("KERNEL_TRACE", "0") == "1"
    res = run_bass_kernel_spmd(nc, in_maps, list(range(NCORES)), trace=trace)
    LAST_RESULTS = res
    LAST_EXEC_NS = res.exec_time_ns

    b = x.shape[0]
    out = np.zeros((b, LAT, DIM), dtype=np.float32)
    for c in range(NCORES):
        out[c // 4] += res.results[c]["y"]
    out += bo[None, None, :]
    return out
